# revision 43
# baseline (speedup 1.0000x reference)
"""Trainium2 Bass kernel for nn_Attention_12146167513140.

Distributed dense attention over 8 NeuronCores.

Sharding: core c in 0..7 -> (b = c//4, head-pair hp = c%4).  Each core
computes the full attention for its 2 heads of its batch, producing a
partial output projection [3072, 256]; the host sums the 4 partials per
batch and adds b_out.

Mask-aware restructuring (the masks are known on the host):
  * k-axis sorted by mask2: valid keys first (KV chunks of 128, zero
    padded), masked keys after (KB chunks).  Masked keys only matter for
    query rows with mask1 == 0 (those take an unmasked softmax since the
    -INF shift is uniform).
  * q-axis sorted by mask1: valid queries (mask1==1) first.  Blocks of
    512 queries that are fully "valid" skip the masked-k chunks
    entirely; fully "free" (mask1==0) blocks use all chunks; the one
    boundary block uses all chunks with a {0,1} column weight applied to
    the masked-chunk exp tiles.
  The per-(q,k) additive mask then never materializes: padded k slots
  are killed via a zeroed "ones" column in the PV matmul.

Engine usage:
  * QK^T: row-tiled (tile_position) 32-row matmuls, 3 chunk-head units
    per PSUM group -> one exp ACTIVATE per [128, 1536] group.
  * PV: col-tiled pairs (both heads of a chunk) accumulating into one
    PSUM bank; a 33rd "ones" column of v accumulates the softmax
    denominator Z.
  * out-projection per 128-q chunk; both heads' unnormalized
    projections and the Z rows ship to the host, which applies the
    1/Z normalization while gathering/summing the per-core partials.
"""

import contextlib
import ctypes
import sys
import types

import numpy as np
import ml_dtypes

import concourse.bacc as bacc
import concourse.mybir as mybir
from concourse import bass_utils
from concourse.tile import TileContext
from concourse.alu_op_type import AluOpType
from concourse.mybir import ActivationFunctionType as AF


def _ensure_trace_support():
    """The container's antenv package lacks axon_hooks; bass_utils
    imports it when tracing is requested (e.g. via BASS_TRACE).  Install
    a functional shim so a traced run works instead of crashing, and
    make the artifact upload a no-op (no bucket access here)."""
    try:
        import antenv.axon_hooks  # noqa: F401
        return
    except ImportError:
        pass
    mod = types.ModuleType("antenv.axon_hooks")
    mod._hook = None
    mod.set_axon_ntff_profile_hook = lambda h: setattr(mod, "_hook", h)
    mod.get_axon_ntff_profile_hook = lambda: mod._hook
    try:
        import antenv
        sys.modules["antenv.axon_hooks"] = mod
        antenv.axon_hooks = mod
    except ImportError:
        sys.modules["antenv.axon_hooks"] = mod

    def _ntff_hook(so_path):
        try:
            lib = ctypes.CDLL(so_path)
        except OSError:
            return None
        if not hasattr(lib, "axon_start_nrt_profile"):
            return None
        lib.axon_start_nrt_profile.argtypes = [ctypes.POINTER(ctypes.c_int64),
                                               ctypes.c_size_t]
        lib.axon_start_nrt_profile.restype = ctypes.c_int64
        lib.axon_stop_nrt_profile.argtypes = [ctypes.c_char_p]
        lib.axon_stop_nrt_profile.restype = ctypes.c_int64

        @contextlib.contextmanager
        def _hook(output_dir, device_ids):
            import jax
            jax.devices()
            if device_ids:
                ids = (ctypes.c_int64 * len(device_ids))(*device_ids)
                rc = lib.axon_start_nrt_profile(ids, len(device_ids))
            else:
                rc = lib.axon_start_nrt_profile(None, 0)
            if rc != 0:
                raise RuntimeError(f"axon_start_nrt_profile rc={rc}")
            try:
                yield
            finally:
                lib.axon_stop_nrt_profile(str(output_dir).encode())

        return _hook

    mod.set_axon_ntff_profile_hook(_ntff_hook("/opt/axon/libaxon_pjrt.so"))

    _orig_upload = bass_utils.upload_artifacts

    def _safe_upload(tmpdir):
        try:
            return _orig_upload(tmpdir)
        except Exception:
            return tmpdir

    bass_utils.upload_artifacts = _safe_upload


_ensure_trace_support()

AX = mybir.AxisListType
I32 = mybir.dt.int32
BF = mybir.dt.bfloat16
F32 = mybir.dt.float32
bf16 = ml_dtypes.bfloat16

B, N1, N2 = 2, 3072, 3072
C_S, H, D = 256, 8, 32
INF = 100000.0
EPS = 1e-8
SCALE = float(np.sqrt(1.0 / (3 * D)))

NCORES = 8
HPC = 2            # heads per core
QCH = N1 // 128    # 24 q row chunks
QB = 512           # q block
NQB = N1 // QB     # 6
VW = D + 1         # 33: v columns + ones column for Z
KUP = 10           # upfront kv chunks (rest JIT during block 0)
QUP = 4            # upfront q chunks (block 0's rows)

_cache = {}


def _build(KV, KT, vb, wblocks, use_g2, scol=None):
    """KV: #valid-k chunks; KT: total k chunks; vb: #pure-valid q blocks;
    wblocks: sorted list of q blocks needing the {0,1} column weight on
    masked-chunk exp tiles."""
    N2P = KT * 128
    NCH = KT + QCH
    nwb = max(1, len(wblocks))

    nc = bacc.Bacc("TRN2", target_bir_lowering=False, debug=False, num_devices=NCORES)

    s1T_d = nc.dram_tensor("s1T", [C_S, N1], BF, kind="ExternalInput")
    s2T_d = nc.dram_tensor("s2T", [C_S, N2P], BF, kind="ExternalInput")
    wq_d = nc.dram_tensor("wq", [C_S, HPC * D], BF, kind="ExternalInput")
    wkv_d = nc.dram_tensor("wkv", [C_S, HPC * 2 * D], BF, kind="ExternalInput")
    wout0_d = nc.dram_tensor("wout0", [D, C_S], BF, kind="ExternalInput")
    wout1_d = nc.dram_tensor("wout1", [D, C_S], BF, kind="ExternalInput")
    vld2_d = nc.dram_tensor("vld2", [128, KT * HPC], BF, kind="ExternalInput")
    wm_d = nc.dram_tensor("wm", [128, nwb * QB], BF, kind="ExternalInput")
    id_d = nc.dram_tensor("ident", [128, 128], BF, kind="ExternalInput")
    if use_g2:
        g2_d = nc.dram_tensor("g2", [128, HPC * D], BF, kind="ExternalInput")
    out_d = nc.dram_tensor("out", [N1, 2 * C_S], BF, kind="ExternalOutput")
    zout_d = nc.dram_tensor("zout", [HPC, N1], BF, kind="ExternalOutput")

    with TileContext(nc) as tc:
        with (
            tc.tile_pool(name="const", bufs=1) as cpool,
            tc.tile_pool(name="norm", bufs=4) as npool,
            tc.tile_pool(name="expp", bufs=4) as expp,
            tc.tile_pool(name="wexp", bufs=2) as wexp,
            tc.tile_pool(name="work", bufs=4) as work,
            tc.tile_pool(name="scp", bufs=2, space="PSUM") as scp,
            tc.tile_pool(name="accp", bufs=1, space="PSUM") as accp,
            tc.tile_pool(name="miscp", bufs=1, space="PSUM") as miscp,
        ):
            # ---- constants / staging (critical-path order) ----
            ident = cpool.tile([128, 128], BF)
            nc.sync.dma_start(ident[:, :], id_d.ap())
            vld2_sb = cpool.tile([128, KT * HPC], BF, tag="vld2")
            nc.sync.dma_start(vld2_sb[:, :], vld2_d.ap())
            # PE heater: dense matmuls through the ~10us initial-DMA ramp +
            # staging so HAM un-throttles to 2.4GHz early and STAYS there.
            # Reads a memset tile (not ident) so it needs no DMA: the burst
            # starts at t~0.  Sized to span until the staged pass1 dummies
            # take over the activity-density job.
            junk = cpool.tile([128, 128], BF, tag="junk")
            nc.vector.memset(junk[:, :], 0.0)
            zrow = cpool.tile([1, 128], BF, tag="zrow")
            nc.vector.memset(zrow[:, :], 0.0)
            heat = accp.tile([128, QB], F32, tag="acc", name="heat")
            for i in range(48):
                nc.tensor.matmul(heat[:, 0:128], junk[:, :], junk[:, :],
                                 start=True, stop=True)
            s1T = [cpool.tile([128, N1], BF, tag=f"s1T{i}", name=f"s1T{i}")
                   for i in range(2)]
            s2T = [cpool.tile([128, N2P], BF, tag=f"s2T{i}", name=f"s2T{i}")
                   for i in range(2)]
            wq_sb = cpool.tile([128, HPC * D], BF, tag="wq")
            wq_sb2 = cpool.tile([128, HPC * D], BF, tag="wq2")
            nc.sync.dma_start(wq_sb[:, :], wq_d.ap()[0:128, :])
            nc.sync.dma_start(wq_sb2[:, :], wq_d.ap()[128:256, :])
            wkv_sb = cpool.tile([128, HPC * 2 * D], BF, tag="wkv")
            wkv_sb2 = cpool.tile([128, HPC * 2 * D], BF, tag="wkv2")
            nc.sync.dma_start(wkv_sb[:, :], wkv_d.ap()[0:128, :])
            nc.sync.dma_start(wkv_sb2[:, :], wkv_d.ap()[128:256, :])
            for i in range(2):
                nc.scalar.dma_start(s1T[i][:, 0:QUP * 128],
                                    s1T_d.ap()[i * 128:(i + 1) * 128,
                                               0:QUP * 128])
            for i in range(2):
                nc.scalar.dma_start(s2T[i][:, 0:KUP * 128],
                                    s2T_d.ap()[i * 128:(i + 1) * 128,
                                               0:KUP * 128])
            # JIT kv tail for block 0 (chunks KUP..KV) right behind the
            # critical slices on the same queue
            for i in range(2):
                nc.scalar.dma_start(s2T[i][:, KUP * 128:(KV + 1) * 128],
                                    s2T_d.ap()[i * 128:(i + 1) * 128,
                                               KUP * 128:(KV + 1) * 128])
            wout0_sb = cpool.tile([D, C_S], BF, tag="wout0")
            wout1_sb = cpool.tile([D, C_S], BF, tag="wout1")
            nc.sync.dma_start(wout0_sb[:, :], wout0_d.ap())
            nc.sync.dma_start(wout1_sb[:, :], wout1_d.ap())
            wm_sb = cpool.tile([128, nwb * QB], BF, tag="wm")
            nc.sync.dma_start(wm_sb[:, :], wm_d.ap())
            if use_g2:
                g2_sb = cpool.tile([128, HPC * D], BF, tag="g2")
                nc.sync.dma_start(g2_sb[:, :], g2_d.ap())

            def dma_rest():
                # bulk input loads, queued after the upfront-critical slices,
                # ordered by consumption: q 4-11, masked kv, q 12-23
                for i in range(2):
                    nc.scalar.dma_start(s1T[i][:, QUP * 128:12 * 128],
                                        s1T_d.ap()[i * 128:(i + 1) * 128,
                                                   QUP * 128:12 * 128])
                for i in range(2):
                    nc.scalar.dma_start(s2T[i][:, (KV + 1) * 128:N2P],
                                        s2T_d.ap()[i * 128:(i + 1) * 128,
                                                   (KV + 1) * 128:N2P])
                for i in range(2):
                    nc.scalar.dma_start(s1T[i][:, 12 * 128:N1],
                                        s1T_d.ap()[i * 128:(i + 1) * 128,
                                                   12 * 128:N1])

            # chunk-pair-packed kT: col block c//2 holds chunks (c even,
            # c odd) at partitions 64*(c%2)+32*h
            KTP = (KT + 1) // 2
            kTp = cpool.tile([128, KTP * 128], BF, tag="kTp")
            # qT: partitions [0:32]=h0, [32:64]=h1, [64:128]=replica
            qTr = cpool.tile([128, N1], BF, tag="qTr")
            vx = cpool.tile([128, KT * HPC * VW], BF, tag="vx")

            def vx_ones():
                # ones columns (zero at k padding slots); emitted AFTER the
                # upfront norms: it waits on the vld2 DMA and would head-of-
                # line block the whole staging DVE chain if emitted first
                nc.vector.tensor_copy(
                    vx[:, :].rearrange("p (n w) -> p n w", w=VW)[:, :, 32:33],
                    vld2_sb[:, :])

            oTz = [cpool.tile([VW, N1], BF, tag=f"oTz{h}", name=f"oTz{h}")
                   for h in range(HPC)]

            kcp_all = cpool.tile([128, NCH * HPC * D], BF, tag="kcp_all")
            ss_all = cpool.tile([128, NCH * HPC], F32, tag="ss_all")
            sr_all = cpool.tile([128, NCH * HPC], F32, tag="sr_all")
            rinv_all = cpool.tile([128, NCH * HPC], F32, tag="rinv_all")

            def pass1g(kcs, sT, w1, w2, kvside, nh=0, jit=False):
                """projection + sumsq for a CONTIGUOUS run of <=4 chunks in
                one PSUM tile: the post-matmul DVE ops (kcp cast, v copy,
                reduce) are each ~150ns fixed overhead, so batch them."""
                ng = len(kcs)
                kc0 = kcs[0]
                ci0 = kc0 if kvside else KT + kc0
                ncol = w1.shape[1]
                xw = ncol // HPC
                if jit:
                    # in-block JIT norms MUST NOT touch the scp pool: its 2
                    # bufs are the QK/exp double-buffer and a third user
                    # stalls the ACT pipeline for the whole tile hold
                    ppf = miscp.tile([128, ng * ncol], F32, tag="mp",
                                     name=f"pp{ci0}")
                else:
                    ppf = scp.tile([128, 1536], F32, tag="sc", name=f"pp{ci0}")
                # staging-phase PE heater dummies: keep activity density high
                # through the preamble so HAM holds K=8/8 (reads junk: no
                # DMA dep, so they fill PE idle while DMA/DVE chains run)
                for _ in range(nh):
                    nc.tensor.matmul(heat[:, 0:128], junk[:, :],
                                     junk[:, :], start=True, stop=True)
                for j, kc in enumerate(kcs):
                    pp = ppf[:, j * ncol:(j + 1) * ncol]
                    nc.tensor.matmul(pp, sT[0][:, kc * 128:(kc + 1) * 128],
                                     w1[:, :], start=True, stop=False)
                    nc.tensor.matmul(pp, sT[1][:, kc * 128:(kc + 1) * 128],
                                     w2[:, :], start=False, stop=True)
                nun = ng * HPC
                pps = ppf[:, 0:ng * ncol].rearrange("p (n x) -> p n x", x=xw)
                kcp = kcp_all[:, ci0 * HPC * D:(ci0 + ng) * HPC * D]
                nc.vector.tensor_copy(
                    kcp.rearrange("p (n d) -> p n d", d=D), pps[:, :, 0:D])
                sq = npool.tile([128, 4 * HPC * D], BF, tag="sq",
                                name=f"sq{ci0}")
                nc.gpsimd.tensor_tensor(sq[:, 0:nun * D], kcp, kcp,
                                        AluOpType.mult)
                nc.vector.reduce_sum(
                    ss_all[:, ci0 * HPC:ci0 * HPC + nun],
                    sq[:, 0:nun * D].rearrange("p (n d) -> p n d", d=D),
                    axis=AX.X)
                if kvside:
                    nc.vector.tensor_copy(
                        vx[:, kc0 * HPC * VW:(kc0 + ng) * HPC * VW]
                        .rearrange("p (n w) -> p n w", w=VW)[:, :, 0:D],
                        pps[:, :, D:2 * D])

            def rsqrt_batch(sl, bid):
                # rinv = 1/sqrt(ss/D + eps) on DVE (bit-trick + 2 Newton
                # steps); dispatch-overhead-bound, so call once per RANGE.
                w = sl.stop - sl.start
                x = sr_all[:, sl]
                nc.vector.tensor_scalar(x, ss_all[:, sl], 1.0 / D, EPS,
                                        AluOpType.mult, AluOpType.add)
                t = npool.tile([128, NCH * HPC], I32, tag="nrt", name=f"nrt{bid}")
                nc.vector.tensor_scalar(t[:, 0:w], x.bitcast(I32), 1, None,
                                        AluOpType.arith_shift_right)
                u = npool.tile([128, NCH * HPC], I32, tag="nru", name=f"nru{bid}")
                nc.vector.tensor_scalar(u[:, 0:w], t[:, 0:w], -1, 0x5F3759DF,
                                        AluOpType.mult, AluOpType.add)
                y = u[:, 0:w].bitcast(F32)
                for it in range(2):
                    a = npool.tile([128, NCH * HPC], F32, tag="nra",
                                   name=f"nra{bid}_{it}")
                    nc.vector.tensor_tensor(a[:, 0:w], y, y, AluOpType.mult)
                    b = npool.tile([128, NCH * HPC], F32, tag="nrb",
                                   name=f"nrb{bid}_{it}")
                    nc.vector.tensor_tensor(b[:, 0:w], a[:, 0:w], x,
                                            AluOpType.mult)
                    c = npool.tile([128, NCH * HPC], F32, tag="nrc",
                                   name=f"nrc{bid}_{it}")
                    nc.vector.tensor_scalar(c[:, 0:w], b[:, 0:w], -0.5, 1.5,
                                            AluOpType.mult, AluOpType.add)
                    dst = (npool.tile([128, NCH * HPC], F32, tag="nry",
                                      name=f"nry{bid}_{it}")
                           if it == 0 else None)
                    out = dst[:, 0:w] if it == 0 else rinv_all[:, sl]
                    nc.vector.tensor_tensor(out, y, c[:, 0:w], AluOpType.mult)
                    y = out

            def pass2(ci0, kc0, qside, npair=2):
                """normalize + transpose a chunk pair (kc0 even, kc0+1), or a
                solo chunk (npair=1).  One PE transpose + 1-2 DVE copies per
                pair instead of per chunk."""
                pre2 = npool.tile([128, 2 * HPC * D], BF, tag="pre",
                                  name=f"pre{ci0}")
                w = npair * HPC * D
                nun = npair * HPC
                # one broadcast TT for the whole pair instead of 4 tiny
                # tensor_scalars: each DVE op pays ~150ns fixed overhead
                rb = (rinv_all[:, ci0 * HPC:ci0 * HPC + nun]
                      .unsqueeze(2).broadcast_to([128, nun, D]))
                nc.vector.tensor_tensor(
                    pre2[:, 0:w].rearrange("p (n d) -> p n d", d=D),
                    kcp_all[:, ci0 * HPC * D:(ci0 + npair) * HPC * D]
                    .rearrange("p (n d) -> p n d", d=D),
                    rb, AluOpType.mult)
                if use_g2 and qside:
                    for j in range(npair):
                        nc.vector.tensor_tensor(
                            pre2[:, 2 * j * D:(2 * j + 2) * D],
                            pre2[:, 2 * j * D:(2 * j + 2) * D],
                            g2_sb[:, :], AluOpType.mult)
                tp2 = miscp.tile([128, 128], BF, tag="mp", name=f"tp{ci0}")
                nc.tensor.transpose(tp2[0:w, :], pre2[:, 0:w], ident[:, :])
                if qside:
                    for j in range(npair):
                        nc.vector.tensor_copy(
                            qTr[0:64, (kc0 + j) * 128:(kc0 + j + 1) * 128],
                            tp2[64 * j:64 * j + 64, :])
                else:
                    nc.vector.tensor_copy(
                        kTp[0:64 * npair, (kc0 // 2) * 128:(kc0 // 2 + 1) * 128],
                        tp2[0:64 * npair, :])

            def norm_batch(chunks, kvside, nh=0):
                for j0 in range(0, len(chunks), 4):
                    sub = chunks[j0:j0 + 4]
                    if kvside:
                        pass1g(sub, s2T, wkv_sb, wkv_sb2, True, nh)
                    else:
                        pass1g(sub, s1T, wq_sb, wq_sb2, False, nh)
                ci0 = (chunks[0] if kvside else KT + chunks[0]) * HPC
                ci1 = (chunks[-1] if kvside else KT + chunks[-1]) * HPC + HPC
                rsqrt_batch(slice(ci0, ci1), f"b{ci0}")
                for kc in chunks[::2]:
                    npair = 2 if kc + 1 in chunks else 1
                    pass2(kc if kvside else KT + kc, kc, not kvside, npair)

            def repl(qb):
                qsl = slice(qb * QB, (qb + 1) * QB)
                nc.vector.tensor_copy(qTr[64:128, qsl], qTr[0:64, qsl])

            def proj_out(qc, tail=False):
                osl = slice(qc * 128, (qc + 1) * 128)
                if tail:
                    op = scp.tile([128, 1536], F32, tag="sc",
                                  name=f"op{qc}")[:, 0:2 * C_S]
                else:
                    op = miscp.tile([128, 2 * C_S], F32, tag="mp",
                                    name=f"op{qc}")
                nc.tensor.matmul(op[:, 0:C_S], oTz[0][0:D, osl], wout0_sb[:, :],
                                 start=True, stop=True)
                nc.tensor.matmul(op[:, C_S:2 * C_S], oTz[1][0:D, osl],
                                 wout1_sb[:, :], start=True, stop=True)
                ob = work.tile([128, 2 * C_S], BF, tag="osb", name=f"osb_{qc}")
                nc.vector.tensor_copy(ob[:, :], op[:, :])
                nc.sync.dma_start(out_d.ap()[osl, :], ob[:, :])

            def attend_block(qb, nch, wslot, fillers=(), nheat=1,
                             mini=None):
                """wslot: None or index into wm_sb blocks for the boundary
                weight applied to masked-chunk (c >= KV) exp tiles."""
                fillers = list(fillers)
                nf = len(fillers)
                qsl = slice(qb * QB, (qb + 1) * QB)
                acc = accp.tile([128, QB], F32, tag="acc", name=f"acc{qb}")
                # Both heads accumulate into disjoint partition ranges of ONE
                # bank.  Two concurrent accumulation groups per bank are
                # illegal, so: zero the bank once (K=1 matmul against a zero
                # row -- keeps the zeroing off the DVE and starts the instant
                # the bank frees), then start=False matmuls (accumulate-onto-
                # zero == overwrite, whatever has_written says).
                nc.tensor.matmul(acc[:, :], zrow[0:1, :], s2T[0][0:1, 0:QB],
                                 start=True, stop=False, skip_group_check=True)
                units = [(c, h) for c in range(nch) for h in range(HPC)]
                nun = len(units)
                ngr = (nun + 2) // 3
                pend = {}
                next_c = [0]

                def flush_pv():
                    while ((next_c[0], 0) in pend and (next_c[0], 1) in pend):
                        c = next_c[0]
                        e0 = pend.pop((c, 0))
                        e1 = pend.pop((c, 1))
                        if wslot is not None and c >= KV:
                            wcol = wm_sb[:, wslot * QB:(wslot + 1) * QB]
                            ew0 = wexp.tile([128, QB], BF, tag="ew0",
                                            name=f"ew0_{qb}_{c}")
                            nc.vector.tensor_tensor(ew0[:, :], e0, wcol,
                                                    AluOpType.mult)
                            e0 = ew0[:, :]
                            ew1 = wexp.tile([128, QB], BF, tag="ew1",
                                            name=f"ew1_{qb}_{c}")
                            nc.vector.tensor_tensor(ew1[:, :], e1, wcol,
                                                    AluOpType.mult)
                            e1 = ew1[:, :]
                        sp = (c == nch - 1) and mini is None
                        nc.tensor.matmul(acc[0:VW, :],
                                         vx[:, (2 * c) * VW:(2 * c + 1) * VW],
                                         e0, start=False, stop=sp,
                                         skip_group_check=True)
                        nc.tensor.matmul(acc[64:64 + VW, :],
                                         vx[:, (2 * c + 1) * VW:(2 * c + 2) * VW],
                                         e1, start=False, stop=sp,
                                         skip_group_check=True)
                        next_c[0] += 1

                for g in range(ngr):
                    grp = units[g * 3:g * 3 + 3]
                    # JIT kv prep MUST be emitted before the group that uses
                    # it (deps are trace-order); pop with 5 groups of
                    # lookahead: a forced pop pulls a whole 4-chunk pass1g
                    # chain (~3us), which must complete before its QK group.
                    la = units[g * 3:g * 3 + 15]
                    cmax = max(c for c, _h in la)
                    while fillers and any(
                            t is not None and t[1] <= cmax for t, _ in fillers):
                        fillers.pop(0)[1]()
                    # pacing: drain all fillers by ~80% of the block so no
                    # tail-flush burst lands at the block transition
                    while fillers and ((nf - len(fillers)) * max(1, ngr - 3)
                                       <= g * nf):
                        fillers.pop(0)[1]()
                    gw = QB * len(grp)
                    sc = scp.tile([128, 1536], F32, tag="sc",
                                  name=f"sc_{qb}_{g}")[:, 0:gw]
                    # full-fat K=M=128 N=512 heat keeps PE stream duty high
                    # so HAM never sees an idle window mid-block.  nheat>0:
                    # that many per group; nheat<0: one every |nheat| groups.
                    nht = nheat if nheat > 0 else (1 if g % -nheat == 0 else 0)
                    for _ in range(nht):
                        nc.tensor.matmul(sc[:, 0:QB], junk[:, :],
                                         s2T[0][:, 0:QB], start=True, stop=True)
                    for j, (c, h) in enumerate(grp):
                        po = 64 * (c % 2) + 32 * h
                        cb = c // 2
                        nc.tensor.matmul(sc[:, j * QB:(j + 1) * QB],
                                         kTp[po:po + 32, cb * 128:(cb + 1) * 128],
                                         qTr[po:po + 32, qsl],
                                         start=True, stop=True,
                                         tile_position=(po, 0))
                    # PVs from the PREVIOUS groups are emitted AFTER this
                    # group's QK: the in-order PE would otherwise stall on a
                    # PV (waiting exp) with the next QK queued behind it,
                    # delaying every exp by the QK latency.
                    flush_pv()
                    ex = expp.tile([128, 1536], BF, tag="ex",
                                   name=f"ex_{qb}_{g}")[:, 0:gw]
                    nc.scalar.activation(ex[:, :], sc[:, :], AF.Exp, scale=SCALE)
                    for j, (c, h) in enumerate(grp):
                        pend[(c, h)] = ex[:, j * QB:(j + 1) * QB]
                while fillers:
                    fillers.pop(0)[1]()
                flush_pv()
                assert not pend and next_c[0] == nch
                if mini is not None:
                    # masked-k chunks only matter for the few stray free-q
                    # rows at the tail of the boundary block: run them on a
                    # narrow column window, weighted by the {0,1} W column.
                    mws, mcol = mini
                    mw = QB - mcol
                    msl = slice(qb * QB + mcol, (qb + 1) * QB)
                    wcol = wm_sb[:, mws * QB + mcol:(mws + 1) * QB]
                    munits = [(c, h) for c in range(KV, KT)
                              for h in range(HPC)]
                    for g0 in range(0, len(munits), 3):
                        grp = munits[g0:g0 + 3]
                        gw = mw * len(grp)
                        msc = scp.tile([128, 1536], F32, tag="sc",
                                       name=f"msc_{g0}")[:, 0:gw]
                        for j, (c, h) in enumerate(grp):
                            po = 64 * (c % 2) + 32 * h
                            cb = c // 2
                            nc.tensor.matmul(
                                msc[:, j * mw:(j + 1) * mw],
                                kTp[po:po + 32, cb * 128:(cb + 1) * 128],
                                qTr[po:po + 32, msl],
                                start=True, stop=True, tile_position=(po, 0))
                        mex = expp.tile([128, 1536], BF, tag="ex",
                                        name=f"mex_{g0}")[:, 0:gw]
                        nc.scalar.activation(mex[:, :], msc[:, :], AF.Exp,
                                             scale=SCALE)
                        for j, (c, h) in enumerate(grp):
                            ew = wexp.tile([128, QB], BF, tag=f"ew{h}",
                                           name=f"mew_{g0}_{j}")[:, 0:mw]
                            nc.vector.tensor_tensor(
                                ew, mex[:, j * mw:(j + 1) * mw], wcol,
                                AluOpType.mult)
                            sp = (c == KT - 1 and h == 1)
                            nc.tensor.matmul(
                                acc[64 * h:64 * h + VW, mcol:QB],
                                vx[:, (2 * c + h) * VW:(2 * c + h + 1) * VW],
                                ew, start=False, stop=sp,
                                skip_group_check=True)

                # block end: unnormalized o + Z row -> sbuf; 1/Z via DRAM bounce
                for h in range(HPC):
                    nc.vector.tensor_copy(oTz[h][:, qsl], acc[64 * h:64 * h + VW, :])
                    nc.sync.dma_start(zout_d.ap()[h:h + 1, qsl],
                                      oTz[h][D:VW, qsl])

            # ---- schedule ----
            # minimal upfront: q chunks for block 0 FIRST (so repl(0) lands
            # early and the bridge burst below can overlap the kv-norm DVE
            # tail), then kv chunks 0..KUP-1.  pass1 heater dummies keep PE
            # dense through the pass1 phase.
            norm_batch(list(range(0, QUP)), kvside=False, nh=5)
            repl(0)
            norm_batch(list(range(0, KUP)), kvside=True, nh=4)
            vx_ones()
            dma_rest()
            # bridge burst: the staging tail (rsqrt/pass2/repl) is DVE-only
            # -- PE would idle >3.4us and HAM would re-throttle right before
            # block 0.  These matmuls read qTr (pinned after repl(0), the
            # last staging op) and are FULL-FAT K=M=N=128: HAM ignores
            # low-occupancy matmuls, so skinny bursts never re-fire SHORT.
            for i in range(70):
                nc.tensor.matmul(heat[:, 0:128], junk[:, :],
                                 qTr[:, 256:384], start=True, stop=True)
            heat_rd = npool.tile([128, 1], F32, tag="heat_rd")
            nc.vector.tensor_copy(heat_rd[:, :], heat[:, 0:1])

            def mkf(fn, *a):
                return lambda: fn(*a)

            def norm_fillers(chunks, kvside):
                """fine-grained fillers through the misc PSUM bank (never the
                QK slots): per-chunk pass1, one rsqrt for the whole range
                (the Newton chain is dispatch-overhead-bound -- batch it),
                per-pair pass2.  kv pass2 fillers are tagged ('kv', c) so
                attend_block can force-emit them before the QK group that
                consumes chunk c."""
                fs = []
                for j0 in range(0, len(chunks), 4):
                    sub = chunks[j0:j0 + 4]
                    if kvside:
                        fs.append((None, mkf(pass1g, sub, s2T, wkv_sb,
                                             wkv_sb2, True, 0, True)))
                        ci0, ci1 = sub[0] * HPC, (sub[-1] + 1) * HPC
                    else:
                        fs.append((None, mkf(pass1g, sub, s1T, wq_sb,
                                             wq_sb2, False, 0, True)))
                        ci0 = (KT + sub[0]) * HPC
                        ci1 = (KT + sub[-1] + 1) * HPC
                    fs.append((None, mkf(rsqrt_batch, slice(ci0, ci1),
                                         f"b{ci0}")))
                    for kc in sub[::2]:
                        npair = 2 if kc + 1 in sub else 1
                        if kvside:
                            fs.append((("kv", kc), mkf(pass2, kc, kc, False,
                                                       npair)))
                        else:
                            fs.append((None, mkf(pass2, KT + kc, kc, True,
                                                 npair)))
                return fs

            # block 0: valid-kv tail + first masked chunk (JIT in block 0),
            # interleaved with block 1's q norm so a kv force-emit also pulls
            # the q chain in early enough
            f0k = norm_fillers(list(range(KUP, KV + 1)), True)
            f0q = (norm_fillers(list(range(QUP, 8)), False) +
                   [(None, mkf(repl, 1))])
            f0 = (f0k[0:4] + f0q[0:4] + f0k[4:8] + f0q[4:] + f0k[8:])
            # block 1: q for block 2, then masked-kv chunks
            f1 = (norm_fillers(list(range(8, 12)), False) +
                  [(None, mkf(repl, 2))] +
                  norm_fillers(list(range(KV + 1, KV + 7)), True))
            # block 2: rest of masked kv (JIT), q for block 3
            f2 = (norm_fillers(list(range(KV + 7, KT)), True) +
                  norm_fillers(list(range(12, 16)), False) +
                  [(None, mkf(repl, 3))])
            # block 3: q for block 4 + first deferred projections
            f3 = (norm_fillers(list(range(16, 20)), False) +
                  [(None, mkf(repl, 4))] +
                  [(None, mkf(proj_out, qc)) for qc in range(0, 8)])
            # block 4: q for block 5 + more projections
            f4 = (norm_fillers(list(range(20, 24)), False) +
                  [(None, mkf(repl, 5))] +
                  [(None, mkf(proj_out, qc)) for qc in range(8, 16)])
            f5 = [(None, mkf(proj_out, qc)) for qc in range(16, 20)]

            wmap = {qb: i for i, qb in enumerate(wblocks)}
            nch_of = lambda qb: KV if qb < vb else KT

            attend_block(0, nch_of(0), wmap.get(0), f0, nheat=2)
            attend_block(1, nch_of(1), wmap.get(1), f1, nheat=2)
            if scol is not None:
                attend_block(2, KV, None, f2, mini=(wmap[2], scol))
            else:
                attend_block(2, nch_of(2), wmap.get(2), f2)
            attend_block(3, nch_of(3), wmap.get(3), f3)
            attend_block(4, nch_of(4), wmap.get(4), f4)
            attend_block(5, nch_of(5), wmap.get(5), f5)
            for qc in range(20, 24):
                proj_out(qc, tail=True)

    nc.compile()
    return nc


def _host_prep(inputs):
    s1 = np.asarray(inputs["s1"], np.float32)
    s2 = np.asarray(inputs["s2"], np.float32)
    ridx1 = np.asarray(inputs["ridx1"], np.int32)
    ct1 = np.asarray(inputs["ct1"], np.int32)
    mask1 = np.asarray(inputs["mask1"], np.int32)
    mask2 = np.asarray(inputs["mask2"], np.int32)
    Wq = np.asarray(inputs["Wq"], np.float32)
    Wkv = np.asarray(inputs["Wkv"], np.float32)
    Wout = np.asarray(inputs["Wout"], np.float32)
    gq = np.asarray(inputs["gq"], np.float32)
    gk = np.asarray(inputs["gk"], np.float32)

    ct_idx = np.take_along_axis(ridx1, ct1[:, None], axis=1)
    pos = (ridx1 - ct_idx).astype(np.float32)
    half = C_S // 2
    freqs = np.exp(-np.log(10000.0) * np.arange(half, dtype=np.float32) / half)
    ang = pos[..., None] * freqs
    s1e = s1 + np.concatenate([np.sin(ang), np.cos(ang)], axis=-1).astype(np.float32)

    g2 = gq * gk
    use_g2 = not np.allclose(g2, 1.0)

    # sort axes by mask
    perm_q = [np.argsort(-mask1[b], kind="stable") for b in range(B)]
    perm_k = [np.argsort(-mask2[b], kind="stable") for b in range(B)]
    nv_q = [int(mask1[b].sum()) for b in range(B)]
    nv_k = [int(mask2[b].sum()) for b in range(B)]

    KV = max((n + 127) // 128 for n in nv_k)
    KB = max((N2 - n + 127) // 128 for n in nv_k)
    KT = KV + KB
    N2P = KT * 128

    vb = min(nv_q) // QB                      # pure-valid q blocks
    wb_end = (max(nv_q) + QB - 1) // QB       # blocks possibly mixed
    wblocks = list(range(vb, wb_end))
    nwb = max(1, len(wblocks))
    # mini-pass window: stray free-q rows at the tail of the boundary block
    # mini-pass disabled: the narrow-window NEFF fails on hardware
    # (CoreSim-clean); boundary block runs all chunks with the W weight.
    scol = None

    ident = np.eye(128, dtype=bf16)
    in_maps = []
    s2p_b, s1p_b, vld_b, wm_b = [], [], [], []
    for b in range(B):
        # padded, sorted k-side
        s2p = np.zeros((N2P, C_S), np.float32)
        s2s = s2[b][perm_k[b]]
        s2p[0:nv_k[b]] = s2s[0:nv_k[b]]
        s2p[KV * 128:KV * 128 + (N2 - nv_k[b])] = s2s[nv_k[b]:]
        valid = np.zeros((N2P,), np.float32)
        valid[0:nv_k[b]] = 1.0
        valid[KV * 128:KV * 128 + (N2 - nv_k[b])] = 1.0
        # vld2 [128, KT*2]: col (2c+h) = valid[c*128 : (c+1)*128]
        vch = valid.reshape(KT, 128).T           # [128, KT]
        vld2 = np.repeat(vch, HPC, axis=1)       # [128, KT*2]
        s1p = s1e[b][perm_q[b]]
        # boundary weights: 1 - mask1 (permuted) per wblock
        m1p = mask1[b][perm_q[b]].astype(np.float32)
        wm = np.zeros((128, nwb * QB), np.float32)
        for i, qb in enumerate(wblocks):
            wm[:, i * QB:(i + 1) * QB] = (
                1.0 - m1p[qb * QB:(qb + 1) * QB])[None, :]
        s2p_b.append(s2p); s1p_b.append(s1p)
        vld_b.append(vld2.astype(bf16)); wm_b.append(wm.astype(bf16))

    for c in range(NCORES):
        b, hp = c // 4, c % 4
        r0 = hp * HPC * D
        m = {
            "s1T": np.ascontiguousarray(s1p_b[b].T).astype(bf16),
            "s2T": np.ascontiguousarray(s2p_b[b].T).astype(bf16),
            "wq": np.ascontiguousarray(Wq[:, hp * HPC * D:(hp + 1) * HPC * D]).astype(bf16),
            "wkv": np.ascontiguousarray(Wkv[:, hp * HPC * 2 * D:(hp + 1) * HPC * 2 * D]).astype(bf16),
            "wout0": np.ascontiguousarray(Wout[r0:r0 + D, :]).astype(bf16),
            "wout1": np.ascontiguousarray(Wout[r0 + D:r0 + 2 * D, :]).astype(bf16),
            "vld2": vld_b[b],
            "wm": wm_b[b],
            "ident": ident,
        }
        if use_g2:
            m["g2"] = np.tile(g2[None, hp * HPC * D:(hp + 1) * HPC * D],
                              (128, 1)).astype(bf16)
        in_maps.append(m)
    params = (KV, KT, vb, tuple(wblocks), use_g2, scol)
    return in_maps, params, perm_q, np.asarray(inputs["b_out"], np.float32)


def _run(inputs, trace=False, **kw):
    in_maps, params, perm_q, b_out = _host_prep(inputs)
    if params not in _cache:
        _cache[params] = _build(params[0], params[1], params[2],
                                list(params[3]), params[4], params[5])
    nc = _cache[params]
    res = bass_utils.run_bass_kernel_spmd(
        nc, in_maps, core_ids=list(range(NCORES)), trace=trace, **kw)
    out = np.zeros((B, N1, C_S), np.float32)
    for c in range(NCORES):
        b = c // 4
        o2 = res.results[c]["out"].astype(np.float32)
        z = res.results[c]["zout"].astype(np.float32)
        part = (o2[:, 0:C_S] / z[0][:, None] + o2[:, C_S:2 * C_S] / z[1][:, None])
        out[b][perm_q[b]] += part
    out += b_out[None, None, :]
    return out, res


def kernel(**inputs) -> np.ndarray:
    out, _ = _run(inputs, trace=False)
    return out



# revision 49
# speedup vs baseline: 1.0940x; 1.0940x over previous
"""Trainium2 Bass kernel for nn_Attention_12146167513140.

Distributed dense attention over 8 NeuronCores.

Sharding: core c in 0..7 -> (b = c//4, head-pair hp = c%4).  Each core
computes the full attention for its 2 heads of its batch, producing a
partial output projection [3072, 256]; the host sums the 4 partials per
batch and adds b_out.

Mask-aware restructuring (the masks are known on the host):
  * k-axis sorted by mask2: valid keys first (KV chunks of 128, zero
    padded), masked keys after (KB chunks).  Masked keys only matter for
    query rows with mask1 == 0 (those take an unmasked softmax since the
    -INF shift is uniform).
  * q-axis sorted by mask1: valid queries (mask1==1) first.  Blocks of
    512 queries that are fully "valid" skip the masked-k chunks
    entirely; fully "free" (mask1==0) blocks use all chunks; the one
    boundary block uses all chunks with a {0,1} column weight applied to
    the masked-chunk exp tiles.
  The per-(q,k) additive mask then never materializes: padded k slots
  are killed via a zeroed "ones" column in the PV matmul.

Engine usage:
  * QK^T: row-tiled (tile_position) 32-row matmuls, 3 chunk-head units
    per PSUM group -> one exp ACTIVATE per [128, 1536] group.
  * PV: col-tiled pairs (both heads of a chunk) accumulating into one
    PSUM bank; a 33rd "ones" column of v accumulates the softmax
    denominator Z.
  * out-projection per 128-q chunk; both heads' unnormalized
    projections and the Z rows ship to the host, which applies the
    1/Z normalization while gathering/summing the per-core partials.
"""

import contextlib
import ctypes
import sys
import types

import numpy as np
import ml_dtypes

import concourse.bacc as bacc
import concourse.mybir as mybir
from concourse import bass_utils
from concourse.tile import TileContext
from concourse.alu_op_type import AluOpType
from concourse.mybir import ActivationFunctionType as AF


def _ensure_trace_support():
    """The container's antenv package lacks axon_hooks; bass_utils
    imports it when tracing is requested (e.g. via BASS_TRACE).  Install
    a functional shim so a traced run works instead of crashing, and
    make the artifact upload a no-op (no bucket access here)."""
    try:
        import antenv.axon_hooks  # noqa: F401
        return
    except ImportError:
        pass
    mod = types.ModuleType("antenv.axon_hooks")
    mod._hook = None
    mod.set_axon_ntff_profile_hook = lambda h: setattr(mod, "_hook", h)
    mod.get_axon_ntff_profile_hook = lambda: mod._hook
    try:
        import antenv
        sys.modules["antenv.axon_hooks"] = mod
        antenv.axon_hooks = mod
    except ImportError:
        sys.modules["antenv.axon_hooks"] = mod

    def _ntff_hook(so_path):
        try:
            lib = ctypes.CDLL(so_path)
        except OSError:
            return None
        if not hasattr(lib, "axon_start_nrt_profile"):
            return None
        lib.axon_start_nrt_profile.argtypes = [ctypes.POINTER(ctypes.c_int64),
                                               ctypes.c_size_t]
        lib.axon_start_nrt_profile.restype = ctypes.c_int64
        lib.axon_stop_nrt_profile.argtypes = [ctypes.c_char_p]
        lib.axon_stop_nrt_profile.restype = ctypes.c_int64

        @contextlib.contextmanager
        def _hook(output_dir, device_ids):
            import jax
            jax.devices()
            if device_ids:
                ids = (ctypes.c_int64 * len(device_ids))(*device_ids)
                rc = lib.axon_start_nrt_profile(ids, len(device_ids))
            else:
                rc = lib.axon_start_nrt_profile(None, 0)
            if rc != 0:
                raise RuntimeError(f"axon_start_nrt_profile rc={rc}")
            try:
                yield
            finally:
                lib.axon_stop_nrt_profile(str(output_dir).encode())

        return _hook

    mod.set_axon_ntff_profile_hook(_ntff_hook("/opt/axon/libaxon_pjrt.so"))

    _orig_upload = bass_utils.upload_artifacts

    def _safe_upload(tmpdir):
        try:
            return _orig_upload(tmpdir)
        except Exception:
            return tmpdir

    bass_utils.upload_artifacts = _safe_upload


_ensure_trace_support()

AX = mybir.AxisListType
I32 = mybir.dt.int32
BF = mybir.dt.bfloat16
F32 = mybir.dt.float32
bf16 = ml_dtypes.bfloat16

B, N1, N2 = 2, 3072, 3072
C_S, H, D = 256, 8, 32
INF = 100000.0
EPS = 1e-8
SCALE = float(np.sqrt(1.0 / (3 * D)))

NCORES = 8
HPC = 2            # heads per core
QCH = N1 // 128    # 24 q row chunks
QB = 512           # q block
NQB = N1 // QB     # 6
VW = D + 1         # 33: v columns + ones column for Z
KUP = 10           # upfront kv chunks (rest JIT during block 0)
QUP = 4            # upfront q chunks (block 0's rows)

_cache = {}


def _build(KV, KT, vb, wblocks, use_g2, scol=None):
    """KV: #valid-k chunks; KT: total k chunks; vb: #pure-valid q blocks;
    wblocks: sorted list of q blocks needing the {0,1} column weight on
    masked-chunk exp tiles."""
    N2P = KT * 128
    NCH = KT + QCH
    nwb = max(1, len(wblocks))

    nc = bacc.Bacc("TRN2", target_bir_lowering=False, debug=False, num_devices=NCORES)

    s1T_d = nc.dram_tensor("s1T", [C_S, N1], BF, kind="ExternalInput")
    s2T_d = nc.dram_tensor("s2T", [C_S, N2P], BF, kind="ExternalInput")
    wq_d = nc.dram_tensor("wq", [C_S, HPC * D], BF, kind="ExternalInput")
    wkv_d = nc.dram_tensor("wkv", [C_S, HPC * 2 * D], BF, kind="ExternalInput")
    wout0_d = nc.dram_tensor("wout0", [D, C_S], BF, kind="ExternalInput")
    wout1_d = nc.dram_tensor("wout1", [D, C_S], BF, kind="ExternalInput")
    vld2_d = nc.dram_tensor("vld2", [128, KT * HPC], BF, kind="ExternalInput")
    wm_d = nc.dram_tensor("wm", [128, nwb * QB], BF, kind="ExternalInput")
    id_d = nc.dram_tensor("ident", [128, 128], BF, kind="ExternalInput")
    if use_g2:
        g2_d = nc.dram_tensor("g2", [128, HPC * D], BF, kind="ExternalInput")
    out_d = nc.dram_tensor("out", [N1, 2 * C_S], BF, kind="ExternalOutput")
    zout_d = nc.dram_tensor("zout", [HPC, N1], BF, kind="ExternalOutput")

    with TileContext(nc) as tc:
        with (
            tc.tile_pool(name="const", bufs=1) as cpool,
            tc.tile_pool(name="norm", bufs=4) as npool,
            tc.tile_pool(name="expp", bufs=4) as expp,
            tc.tile_pool(name="wexp", bufs=2) as wexp,
            tc.tile_pool(name="work", bufs=4) as work,
            tc.tile_pool(name="scp", bufs=2, space="PSUM") as scp,
            tc.tile_pool(name="accp", bufs=1, space="PSUM") as accp,
            tc.tile_pool(name="miscp", bufs=1, space="PSUM") as miscp,
        ):
            # ---- constants / staging (critical-path order) ----
            ident = cpool.tile([128, 128], BF)
            nc.sync.dma_start(ident[:, :], id_d.ap())
            vld2_sb = cpool.tile([128, KT * HPC], BF, tag="vld2")
            nc.sync.dma_start(vld2_sb[:, :], vld2_d.ap())
            # PE heater: dense matmuls through the ~10us initial-DMA ramp +
            # staging so HAM un-throttles to 2.4GHz early and STAYS there.
            # Reads a memset tile (not ident) so it needs no DMA: the burst
            # starts at t~0.  Sized to span until the staged pass1 dummies
            # take over the activity-density job.
            junk = cpool.tile([128, 128], BF, tag="junk")
            nc.vector.memset(junk[:, :], 0.0)
            zrow = cpool.tile([1, 128], BF, tag="zrow")
            nc.vector.memset(zrow[:, :], 0.0)
            heat = accp.tile([128, QB], F32, tag="acc", name="heat")
            for i in range(48):
                nc.tensor.matmul(heat[:, 0:128], junk[:, :], junk[:, :],
                                 start=True, stop=True)
            s1T = [cpool.tile([128, N1], BF, tag=f"s1T{i}", name=f"s1T{i}")
                   for i in range(2)]
            s2T = [cpool.tile([128, N2P], BF, tag=f"s2T{i}", name=f"s2T{i}")
                   for i in range(2)]
            wq_sb = cpool.tile([128, HPC * D], BF, tag="wq")
            wq_sb2 = cpool.tile([128, HPC * D], BF, tag="wq2")
            nc.sync.dma_start(wq_sb[:, :], wq_d.ap()[0:128, :])
            nc.sync.dma_start(wq_sb2[:, :], wq_d.ap()[128:256, :])
            wkv_sb = cpool.tile([128, HPC * 2 * D], BF, tag="wkv")
            wkv_sb2 = cpool.tile([128, HPC * 2 * D], BF, tag="wkv2")
            nc.sync.dma_start(wkv_sb[:, :], wkv_d.ap()[0:128, :])
            nc.sync.dma_start(wkv_sb2[:, :], wkv_d.ap()[128:256, :])
            for i in range(2):
                nc.scalar.dma_start(s1T[i][:, 0:QUP * 128],
                                    s1T_d.ap()[i * 128:(i + 1) * 128,
                                               0:QUP * 128])
            for i in range(2):
                nc.scalar.dma_start(s2T[i][:, 0:KUP * 128],
                                    s2T_d.ap()[i * 128:(i + 1) * 128,
                                               0:KUP * 128])
            # JIT kv tail for block 0 (chunks KUP..KV) right behind the
            # critical slices on the same queue
            for i in range(2):
                nc.scalar.dma_start(s2T[i][:, KUP * 128:(KV + 1) * 128],
                                    s2T_d.ap()[i * 128:(i + 1) * 128,
                                               KUP * 128:(KV + 1) * 128])
            wout0_sb = cpool.tile([D, C_S], BF, tag="wout0")
            wout1_sb = cpool.tile([D, C_S], BF, tag="wout1")
            nc.sync.dma_start(wout0_sb[:, :], wout0_d.ap())
            nc.sync.dma_start(wout1_sb[:, :], wout1_d.ap())
            wm_sb = cpool.tile([128, nwb * QB], BF, tag="wm")
            nc.sync.dma_start(wm_sb[:, :], wm_d.ap())
            if use_g2:
                g2_sb = cpool.tile([128, HPC * D], BF, tag="g2")
                nc.sync.dma_start(g2_sb[:, :], g2_d.ap())

            def dma_rest():
                # bulk input loads, queued after the upfront-critical slices,
                # ordered by consumption: q 4-11, masked kv, q 12-23
                for i in range(2):
                    nc.scalar.dma_start(s1T[i][:, QUP * 128:12 * 128],
                                        s1T_d.ap()[i * 128:(i + 1) * 128,
                                                   QUP * 128:12 * 128])
                for i in range(2):
                    nc.scalar.dma_start(s2T[i][:, (KV + 1) * 128:N2P],
                                        s2T_d.ap()[i * 128:(i + 1) * 128,
                                                   (KV + 1) * 128:N2P])
                for i in range(2):
                    nc.scalar.dma_start(s1T[i][:, 12 * 128:N1],
                                        s1T_d.ap()[i * 128:(i + 1) * 128,
                                                   12 * 128:N1])

            # chunk-pair-packed kT: col block c//2 holds chunks (c even,
            # c odd) at partitions 64*(c%2)+32*h
            KTP = (KT + 1) // 2
            kTp = cpool.tile([128, KTP * 128], BF, tag="kTp")
            # qT: partitions [0:32]=h0, [32:64]=h1, [64:128]=replica
            qTr = cpool.tile([128, N1], BF, tag="qTr")
            vx = cpool.tile([128, KT * HPC * VW], BF, tag="vx")

            def vx_ones():
                # ones columns (zero at k padding slots); emitted AFTER the
                # upfront norms: it waits on the vld2 DMA and would head-of-
                # line block the whole staging DVE chain if emitted first
                nc.vector.tensor_copy(
                    vx[:, :].rearrange("p (n w) -> p n w", w=VW)[:, :, 32:33],
                    vld2_sb[:, :])

            oTz = [cpool.tile([VW, N1], BF, tag=f"oTz{h}", name=f"oTz{h}")
                   for h in range(HPC)]

            kcp_all = cpool.tile([128, NCH * HPC * D], BF, tag="kcp_all")
            ss_all = cpool.tile([128, NCH * HPC], F32, tag="ss_all")
            sr_all = cpool.tile([128, NCH * HPC], F32, tag="sr_all")
            rinv_all = cpool.tile([128, NCH * HPC], F32, tag="rinv_all")

            def pass1g(kcs, sT, w1, w2, kvside, nh=0, jit=False):
                """projection + sumsq for a CONTIGUOUS run of <=4 chunks in
                one PSUM tile: the post-matmul DVE ops (kcp cast, v copy,
                reduce) are each ~150ns fixed overhead, so batch them."""
                ng = len(kcs)
                kc0 = kcs[0]
                ci0 = kc0 if kvside else KT + kc0
                ncol = w1.shape[1]
                xw = ncol // HPC
                if jit:
                    # in-block JIT norms MUST NOT touch the scp pool: its 2
                    # bufs are the QK/exp double-buffer and a third user
                    # stalls the ACT pipeline for the whole tile hold
                    ppf = miscp.tile([128, ng * ncol], F32, tag="mp",
                                     name=f"pp{ci0}")
                else:
                    ppf = scp.tile([128, 1536], F32, tag="sc", name=f"pp{ci0}")
                # staging-phase PE heater dummies: keep activity density high
                # through the preamble so HAM holds K=8/8 (reads junk: no
                # DMA dep, so they fill PE idle while DMA/DVE chains run)
                for _ in range(nh):
                    nc.tensor.matmul(heat[:, 0:128], junk[:, :],
                                     junk[:, :], start=True, stop=True)
                for j, kc in enumerate(kcs):
                    pp = ppf[:, j * ncol:(j + 1) * ncol]
                    nc.tensor.matmul(pp, sT[0][:, kc * 128:(kc + 1) * 128],
                                     w1[:, :], start=True, stop=False)
                    nc.tensor.matmul(pp, sT[1][:, kc * 128:(kc + 1) * 128],
                                     w2[:, :], start=False, stop=True)
                nun = ng * HPC
                pps = ppf[:, 0:ng * ncol].rearrange("p (n x) -> p n x", x=xw)
                kcp = kcp_all[:, ci0 * HPC * D:(ci0 + ng) * HPC * D]
                nc.vector.tensor_copy(
                    kcp.rearrange("p (n d) -> p n d", d=D), pps[:, :, 0:D])
                sq = npool.tile([128, 4 * HPC * D], BF, tag="sq",
                                name=f"sq{ci0}")
                nc.gpsimd.tensor_tensor(sq[:, 0:nun * D], kcp, kcp,
                                        AluOpType.mult)
                nc.vector.reduce_sum(
                    ss_all[:, ci0 * HPC:ci0 * HPC + nun],
                    sq[:, 0:nun * D].rearrange("p (n d) -> p n d", d=D),
                    axis=AX.X)
                if kvside:
                    nc.vector.tensor_copy(
                        vx[:, kc0 * HPC * VW:(kc0 + ng) * HPC * VW]
                        .rearrange("p (n w) -> p n w", w=VW)[:, :, 0:D],
                        pps[:, :, D:2 * D])

            def rsqrt_batch(sl, bid):
                # rinv = 1/sqrt(ss/D + eps) on DVE (bit-trick + 2 Newton
                # steps); dispatch-overhead-bound, so call once per RANGE.
                w = sl.stop - sl.start
                x = sr_all[:, sl]
                nc.vector.tensor_scalar(x, ss_all[:, sl], 1.0 / D, EPS,
                                        AluOpType.mult, AluOpType.add)
                t = npool.tile([128, NCH * HPC], I32, tag="nrt", name=f"nrt{bid}")
                nc.vector.tensor_scalar(t[:, 0:w], x.bitcast(I32), 1, None,
                                        AluOpType.arith_shift_right)
                u = npool.tile([128, NCH * HPC], I32, tag="nru", name=f"nru{bid}")
                nc.vector.tensor_scalar(u[:, 0:w], t[:, 0:w], -1, 0x5F3759DF,
                                        AluOpType.mult, AluOpType.add)
                y = u[:, 0:w].bitcast(F32)
                for it in range(2):
                    a = npool.tile([128, NCH * HPC], F32, tag="nra",
                                   name=f"nra{bid}_{it}")
                    nc.vector.tensor_tensor(a[:, 0:w], y, y, AluOpType.mult)
                    b = npool.tile([128, NCH * HPC], F32, tag="nrb",
                                   name=f"nrb{bid}_{it}")
                    nc.vector.tensor_tensor(b[:, 0:w], a[:, 0:w], x,
                                            AluOpType.mult)
                    c = npool.tile([128, NCH * HPC], F32, tag="nrc",
                                   name=f"nrc{bid}_{it}")
                    nc.vector.tensor_scalar(c[:, 0:w], b[:, 0:w], -0.5, 1.5,
                                            AluOpType.mult, AluOpType.add)
                    dst = (npool.tile([128, NCH * HPC], F32, tag="nry",
                                      name=f"nry{bid}_{it}")
                           if it == 0 else None)
                    out = dst[:, 0:w] if it == 0 else rinv_all[:, sl]
                    nc.vector.tensor_tensor(out, y, c[:, 0:w], AluOpType.mult)
                    y = out

            def pass2(ci0, kc0, qside, npair=2):
                """normalize + transpose a chunk pair (kc0 even, kc0+1), or a
                solo chunk (npair=1).  One PE transpose + 1-2 DVE copies per
                pair instead of per chunk."""
                pre2 = npool.tile([128, 2 * HPC * D], BF, tag="pre",
                                  name=f"pre{ci0}")
                w = npair * HPC * D
                nun = npair * HPC
                # one broadcast TT for the whole pair instead of 4 tiny
                # tensor_scalars: each DVE op pays ~150ns fixed overhead
                rb = (rinv_all[:, ci0 * HPC:ci0 * HPC + nun]
                      .unsqueeze(2).broadcast_to([128, nun, D]))
                nc.vector.tensor_tensor(
                    pre2[:, 0:w].rearrange("p (n d) -> p n d", d=D),
                    kcp_all[:, ci0 * HPC * D:(ci0 + npair) * HPC * D]
                    .rearrange("p (n d) -> p n d", d=D),
                    rb, AluOpType.mult)
                if use_g2 and qside:
                    for j in range(npair):
                        nc.vector.tensor_tensor(
                            pre2[:, 2 * j * D:(2 * j + 2) * D],
                            pre2[:, 2 * j * D:(2 * j + 2) * D],
                            g2_sb[:, :], AluOpType.mult)
                tp2 = miscp.tile([128, 128], BF, tag="mp", name=f"tp{ci0}")
                nc.tensor.transpose(tp2[0:w, :], pre2[:, 0:w], ident[:, :])
                if qside:
                    for j in range(npair):
                        nc.vector.tensor_copy(
                            qTr[0:64, (kc0 + j) * 128:(kc0 + j + 1) * 128],
                            tp2[64 * j:64 * j + 64, :])
                else:
                    nc.vector.tensor_copy(
                        kTp[0:64 * npair, (kc0 // 2) * 128:(kc0 // 2 + 1) * 128],
                        tp2[0:64 * npair, :])

            def norm_batch(chunks, kvside, nh=0):
                for j0 in range(0, len(chunks), 4):
                    sub = chunks[j0:j0 + 4]
                    if kvside:
                        pass1g(sub, s2T, wkv_sb, wkv_sb2, True, nh)
                    else:
                        pass1g(sub, s1T, wq_sb, wq_sb2, False, nh)
                ci0 = (chunks[0] if kvside else KT + chunks[0]) * HPC
                ci1 = (chunks[-1] if kvside else KT + chunks[-1]) * HPC + HPC
                rsqrt_batch(slice(ci0, ci1), f"b{ci0}")
                for kc in chunks[::2]:
                    npair = 2 if kc + 1 in chunks else 1
                    pass2(kc if kvside else KT + kc, kc, not kvside, npair)

            def repl(qb):
                qsl = slice(qb * QB, (qb + 1) * QB)
                nc.vector.tensor_copy(qTr[64:128, qsl], qTr[0:64, qsl])

            def proj_out(qc, tail=False):
                osl = slice(qc * 128, (qc + 1) * 128)
                if tail:
                    op = scp.tile([128, 1536], F32, tag="sc",
                                  name=f"op{qc}")[:, 0:2 * C_S]
                else:
                    op = miscp.tile([128, 2 * C_S], F32, tag="mp",
                                    name=f"op{qc}")
                nc.tensor.matmul(op[:, 0:C_S], oTz[0][0:D, osl], wout0_sb[:, :],
                                 start=True, stop=True)
                nc.tensor.matmul(op[:, C_S:2 * C_S], oTz[1][0:D, osl],
                                 wout1_sb[:, :], start=True, stop=True)
                ob = work.tile([128, 2 * C_S], BF, tag="osb", name=f"osb_{qc}")
                nc.vector.tensor_copy(ob[:, :], op[:, :])
                nc.sync.dma_start(out_d.ap()[osl, :], ob[:, :])

            def attend_block(q0, w, nch, wslot, fillers=(), nheat=1,
                             woff=0):
                """Attend q columns [q0, q0+w) against chunks 0..nch-1.
                wslot: None or index into wm_sb blocks for the boundary
                weight applied to masked-chunk (c >= KV) exp tiles; woff is
                the column offset of this sub-block inside its wm block."""
                bid = f"{q0 // 128}w{w}"
                upg = 1536 // w            # units per 1536-col PSUM tile
                fillers = list(fillers)
                nf = len(fillers)
                qsl = slice(q0, q0 + w)
                acc = accp.tile([128, QB], F32, tag="acc", name=f"acc{bid}")
                # Both heads accumulate into disjoint partition ranges of ONE
                # bank.  Two concurrent accumulation groups per bank are
                # illegal, so: zero the bank once (K=1 matmul against a zero
                # row -- keeps the zeroing off the DVE and starts the instant
                # the bank frees), then start=False matmuls (accumulate-onto-
                # zero == overwrite, whatever has_written says).
                nc.tensor.matmul(acc[:, 0:w], zrow[0:1, :], s2T[0][0:1, 0:w],
                                 start=True, stop=False, skip_group_check=True)
                units = [(c, h) for c in range(nch) for h in range(HPC)]
                nun = len(units)
                ngr = (nun + upg - 1) // upg
                pend = {}
                next_c = [0]

                def flush_pv():
                    while ((next_c[0], 0) in pend and (next_c[0], 1) in pend):
                        c = next_c[0]
                        e0 = pend.pop((c, 0))
                        e1 = pend.pop((c, 1))
                        if wslot is not None and c >= KV:
                            wcol = wm_sb[:, wslot * QB + woff:
                                         wslot * QB + woff + w]
                            ew0 = wexp.tile([128, QB], BF, tag="ew0",
                                            name=f"ew0_{bid}_{c}")
                            nc.vector.tensor_tensor(ew0[:, 0:w], e0, wcol,
                                                    AluOpType.mult)
                            e0 = ew0[:, 0:w]
                            ew1 = wexp.tile([128, QB], BF, tag="ew1",
                                            name=f"ew1_{bid}_{c}")
                            nc.vector.tensor_tensor(ew1[:, 0:w], e1, wcol,
                                                    AluOpType.mult)
                            e1 = ew1[:, 0:w]
                        sp = (c == nch - 1)
                        nc.tensor.matmul(acc[0:VW, 0:w],
                                         vx[:, (2 * c) * VW:(2 * c + 1) * VW],
                                         e0, start=False, stop=sp,
                                         skip_group_check=True)
                        nc.tensor.matmul(acc[64:64 + VW, 0:w],
                                         vx[:, (2 * c + 1) * VW:(2 * c + 2) * VW],
                                         e1, start=False, stop=sp,
                                         skip_group_check=True)
                        next_c[0] += 1

                for g in range(ngr):
                    grp = units[g * upg:(g + 1) * upg]
                    # JIT kv prep MUST be emitted before the group that uses
                    # it (deps are trace-order); pop with 5 groups of
                    # lookahead: a forced pop pulls a whole 4-chunk pass1g
                    # chain (~3us), which must complete before its QK group.
                    la = units[g * upg:g * upg + 5 * upg]
                    cmax = max(c for c, _h in la)
                    while fillers and any(
                            t is not None and t[1] <= cmax for t, _ in fillers):
                        fillers.pop(0)[1]()
                    # pacing: drain all fillers by ~80% of the block so no
                    # tail-flush burst lands at the block transition
                    while fillers and ((nf - len(fillers)) * max(1, ngr - 3)
                                       <= g * nf):
                        fillers.pop(0)[1]()
                    gw = w * len(grp)
                    scf = scp.tile([128, 1536], F32, tag="sc",
                                   name=f"sc_{bid}_{g}")
                    sc = scf[:, 0:gw]
                    # full-fat K=M=128 N=512 heat keeps PE stream duty high
                    # so HAM never sees an idle window mid-block.  nheat>0:
                    # that many per group; nheat<0: one every |nheat| groups.
                    nht = nheat if nheat > 0 else (1 if g % -nheat == 0 else 0)
                    for _ in range(nht):
                        nc.tensor.matmul(scf[:, 1024:1536], junk[:, :],
                                         s2T[0][:, 0:QB], start=True, stop=True)
                    for j, (c, h) in enumerate(grp):
                        po = 64 * (c % 2) + 32 * h
                        cb = c // 2
                        nc.tensor.matmul(sc[:, j * w:(j + 1) * w],
                                         kTp[po:po + 32, cb * 128:(cb + 1) * 128],
                                         qTr[po:po + 32, qsl],
                                         start=True, stop=True,
                                         tile_position=(po, 0))
                    # PVs from the PREVIOUS groups are emitted AFTER this
                    # group's QK: the in-order PE would otherwise stall on a
                    # PV (waiting exp) with the next QK queued behind it,
                    # delaying every exp by the QK latency.
                    flush_pv()
                    ex = expp.tile([128, 1536], BF, tag="ex",
                                   name=f"ex_{bid}_{g}")[:, 0:gw]
                    nc.scalar.activation(ex[:, :], sc[:, :], AF.Exp, scale=SCALE)
                    for j, (c, h) in enumerate(grp):
                        pend[(c, h)] = ex[:, j * w:(j + 1) * w]
                while fillers:
                    fillers.pop(0)[1]()
                flush_pv()
                assert not pend and next_c[0] == nch

                # block end: unnormalized o + Z row -> sbuf; 1/Z via DRAM bounce
                for h in range(HPC):
                    nc.vector.tensor_copy(oTz[h][:, qsl],
                                          acc[64 * h:64 * h + VW, 0:w])
                    nc.sync.dma_start(zout_d.ap()[h:h + 1, qsl],
                                      oTz[h][D:VW, qsl])

            # ---- schedule ----
            # minimal upfront: q chunks for block 0 FIRST (so repl(0) lands
            # early and the bridge burst below can overlap the kv-norm DVE
            # tail), then kv chunks 0..KUP-1.  pass1 heater dummies keep PE
            # dense through the pass1 phase.
            norm_batch(list(range(0, QUP)), kvside=False, nh=5)
            repl(0)
            norm_batch(list(range(0, KUP)), kvside=True, nh=4)
            vx_ones()
            dma_rest()
            # bridge burst: the staging tail (rsqrt/pass2/repl) is DVE-only
            # -- PE would idle >3.4us and HAM would re-throttle right before
            # block 0.  These matmuls read qTr (pinned after repl(0), the
            # last staging op) and are FULL-FAT K=M=N=128: HAM ignores
            # low-occupancy matmuls, so skinny bursts never re-fire SHORT.
            for i in range(70):
                nc.tensor.matmul(heat[:, 0:128], junk[:, :],
                                 qTr[:, 256:384], start=True, stop=True)
            heat_rd = npool.tile([128, 1], F32, tag="heat_rd")
            nc.vector.tensor_copy(heat_rd[:, :], heat[:, 0:1])

            def mkf(fn, *a):
                return lambda: fn(*a)

            def norm_fillers(chunks, kvside):
                """fine-grained fillers through the misc PSUM bank (never the
                QK slots): per-chunk pass1, one rsqrt for the whole range
                (the Newton chain is dispatch-overhead-bound -- batch it),
                per-pair pass2.  kv pass2 fillers are tagged ('kv', c) so
                attend_block can force-emit them before the QK group that
                consumes chunk c."""
                fs = []
                for j0 in range(0, len(chunks), 4):
                    sub = chunks[j0:j0 + 4]
                    if kvside:
                        fs.append((None, mkf(pass1g, sub, s2T, wkv_sb,
                                             wkv_sb2, True, 0, True)))
                        ci0, ci1 = sub[0] * HPC, (sub[-1] + 1) * HPC
                    else:
                        fs.append((None, mkf(pass1g, sub, s1T, wq_sb,
                                             wq_sb2, False, 0, True)))
                        ci0 = (KT + sub[0]) * HPC
                        ci1 = (KT + sub[-1] + 1) * HPC
                    fs.append((None, mkf(rsqrt_batch, slice(ci0, ci1),
                                         f"b{ci0}")))
                    for kc in sub[::2]:
                        npair = 2 if kc + 1 in sub else 1
                        if kvside:
                            fs.append((("kv", kc), mkf(pass2, kc, kc, False,
                                                       npair)))
                        else:
                            fs.append((None, mkf(pass2, KT + kc, kc, True,
                                                 npair)))
                return fs

            # block 0: valid-kv tail + first masked chunk (JIT in block 0),
            # interleaved with block 1's q norm so a kv force-emit also pulls
            # the q chain in early enough
            f0k = norm_fillers(list(range(KUP, KV + 1)), True)
            f0q = (norm_fillers(list(range(QUP, 8)), False) +
                   [(None, mkf(repl, 1))])
            f0 = (f0k[0:4] + f0q[0:4] + f0k[4:8] + f0q[4:] + f0k[8:])
            # block 1: q for block 2, then masked-kv chunks
            f1 = (norm_fillers(list(range(8, 12)), False) +
                  [(None, mkf(repl, 2))] +
                  norm_fillers(list(range(KV + 1, KV + 7)), True))
            # block 2: rest of masked kv (JIT), q for block 3
            f2 = (norm_fillers(list(range(KV + 7, KT)), True) +
                  norm_fillers(list(range(12, 16)), False) +
                  [(None, mkf(repl, 3))])
            # block 3: q for block 4 + first deferred projections
            f3 = (norm_fillers(list(range(16, 20)), False) +
                  [(None, mkf(repl, 4))] +
                  [(None, mkf(proj_out, qc)) for qc in range(0, 8)])
            # block 4: q for block 5 + more projections
            f4 = (norm_fillers(list(range(20, 24)), False) +
                  [(None, mkf(repl, 5))] +
                  [(None, mkf(proj_out, qc)) for qc in range(8, 16)])
            f5 = [(None, mkf(proj_out, qc)) for qc in range(16, 20)]

            wmap = {qb: i for i, qb in enumerate(wblocks)}
            nch_of = lambda qb: KV if qb < vb else KT

            attend_block(0, QB, nch_of(0), wmap.get(0), f0, nheat=2)
            attend_block(QB, QB, nch_of(1), wmap.get(1), f1, nheat=2)
            if scol is not None and 2 in wmap:
                # boundary block split: the free-q rows all sit in the last
                # QB-scol columns, so the leading scol columns take the
                # cheap valid-only path and only the narrow tail runs all
                # chunks (with the W weight killing the valid rows' masked-
                # chunk terms).
                attend_block(2 * QB, scol, KV, None, f2)
                attend_block(2 * QB + scol, QB - scol, KT, wmap[2], [],
                             woff=scol)
            else:
                attend_block(2 * QB, QB, nch_of(2), wmap.get(2), f2)
            attend_block(3 * QB, QB, nch_of(3), wmap.get(3), f3)
            attend_block(4 * QB, QB, nch_of(4), wmap.get(4), f4)
            attend_block(5 * QB, QB, nch_of(5), wmap.get(5), f5)
            for qc in range(20, 24):
                proj_out(qc, tail=True)

    nc.compile()
    return nc


def _host_prep(inputs):
    s1 = np.asarray(inputs["s1"], np.float32)
    s2 = np.asarray(inputs["s2"], np.float32)
    ridx1 = np.asarray(inputs["ridx1"], np.int32)
    ct1 = np.asarray(inputs["ct1"], np.int32)
    mask1 = np.asarray(inputs["mask1"], np.int32)
    mask2 = np.asarray(inputs["mask2"], np.int32)
    Wq = np.asarray(inputs["Wq"], np.float32)
    Wkv = np.asarray(inputs["Wkv"], np.float32)
    Wout = np.asarray(inputs["Wout"], np.float32)
    gq = np.asarray(inputs["gq"], np.float32)
    gk = np.asarray(inputs["gk"], np.float32)

    ct_idx = np.take_along_axis(ridx1, ct1[:, None], axis=1)
    pos = (ridx1 - ct_idx).astype(np.float32)
    half = C_S // 2
    freqs = np.exp(-np.log(10000.0) * np.arange(half, dtype=np.float32) / half)
    ang = pos[..., None] * freqs
    s1e = s1 + np.concatenate([np.sin(ang), np.cos(ang)], axis=-1).astype(np.float32)

    g2 = gq * gk
    use_g2 = not np.allclose(g2, 1.0)

    # sort axes by mask
    perm_q = [np.argsort(-mask1[b], kind="stable") for b in range(B)]
    perm_k = [np.argsort(-mask2[b], kind="stable") for b in range(B)]
    nv_q = [int(mask1[b].sum()) for b in range(B)]
    nv_k = [int(mask2[b].sum()) for b in range(B)]

    KV = max((n + 127) // 128 for n in nv_k)
    KB = max((N2 - n + 127) // 128 for n in nv_k)
    KT = KV + KB
    N2P = KT * 128

    vb = min(nv_q) // QB                      # pure-valid q blocks
    wb_end = (max(nv_q) + QB - 1) // QB       # blocks possibly mixed
    wblocks = list(range(vb, wb_end))
    nwb = max(1, len(wblocks))
    # mini-pass window: stray free-q rows at the tail of the boundary block.
    # A truly narrow window (mw ~ #stray rows) produced a NEFF that fails on
    # hardware, so use a fixed 128-wide window (standard shapes): masked-k
    # chunks run over the last 128 q columns only, with the W weight zeroing
    # the valid-row columns inside the window.
    # Narrow-window attend (mini-pass or a split boundary block) produces
    # NEFFs that fail at runtime on hardware (CoreSim-clean) -- verified for
    # window widths 9, 128, 256.  The boundary block therefore runs all
    # chunks at full width with the W weight on masked-chunk exp tiles.
    scol = None

    ident = np.eye(128, dtype=bf16)
    in_maps = []
    s2p_b, s1p_b, vld_b, wm_b = [], [], [], []
    for b in range(B):
        # padded, sorted k-side
        s2p = np.zeros((N2P, C_S), np.float32)
        s2s = s2[b][perm_k[b]]
        s2p[0:nv_k[b]] = s2s[0:nv_k[b]]
        s2p[KV * 128:KV * 128 + (N2 - nv_k[b])] = s2s[nv_k[b]:]
        valid = np.zeros((N2P,), np.float32)
        valid[0:nv_k[b]] = 1.0
        valid[KV * 128:KV * 128 + (N2 - nv_k[b])] = 1.0
        # vld2 [128, KT*2]: col (2c+h) = valid[c*128 : (c+1)*128]
        vch = valid.reshape(KT, 128).T           # [128, KT]
        vld2 = np.repeat(vch, HPC, axis=1)       # [128, KT*2]
        s1p = s1e[b][perm_q[b]]
        # boundary weights: 1 - mask1 (permuted) per wblock
        m1p = mask1[b][perm_q[b]].astype(np.float32)
        wm = np.zeros((128, nwb * QB), np.float32)
        for i, qb in enumerate(wblocks):
            wm[:, i * QB:(i + 1) * QB] = (
                1.0 - m1p[qb * QB:(qb + 1) * QB])[None, :]
        s2p_b.append(s2p); s1p_b.append(s1p)
        vld_b.append(vld2.astype(bf16)); wm_b.append(wm.astype(bf16))

    for c in range(NCORES):
        b, hp = c // 4, c % 4
        r0 = hp * HPC * D
        m = {
            "s1T": np.ascontiguousarray(s1p_b[b].T).astype(bf16),
            "s2T": np.ascontiguousarray(s2p_b[b].T).astype(bf16),
            "wq": np.ascontiguousarray(Wq[:, hp * HPC * D:(hp + 1) * HPC * D]).astype(bf16),
            "wkv": np.ascontiguousarray(Wkv[:, hp * HPC * 2 * D:(hp + 1) * HPC * 2 * D]).astype(bf16),
            "wout0": np.ascontiguousarray(Wout[r0:r0 + D, :]).astype(bf16),
            "wout1": np.ascontiguousarray(Wout[r0 + D:r0 + 2 * D, :]).astype(bf16),
            "vld2": vld_b[b],
            "wm": wm_b[b],
            "ident": ident,
        }
        if use_g2:
            m["g2"] = np.tile(g2[None, hp * HPC * D:(hp + 1) * HPC * D],
                              (128, 1)).astype(bf16)
        in_maps.append(m)
    params = (KV, KT, vb, tuple(wblocks), use_g2, scol)
    return in_maps, params, perm_q, np.asarray(inputs["b_out"], np.float32)


def _run(inputs, trace=False, **kw):
    in_maps, params, perm_q, b_out = _host_prep(inputs)
    if params not in _cache:
        _cache[params] = _build(params[0], params[1], params[2],
                                list(params[3]), params[4], params[5])
    nc = _cache[params]
    res = bass_utils.run_bass_kernel_spmd(
        nc, in_maps, core_ids=list(range(NCORES)), trace=trace, **kw)
    out = np.zeros((B, N1, C_S), np.float32)
    for c in range(NCORES):
        b = c // 4
        o2 = res.results[c]["out"].astype(np.float32)
        z = res.results[c]["zout"].astype(np.float32)
        part = (o2[:, 0:C_S] / z[0][:, None] + o2[:, C_S:2 * C_S] / z[1][:, None])
        out[b][perm_q[b]] += part
    out += b_out[None, None, :]
    return out, res


def kernel(**inputs) -> np.ndarray:
    out, _ = _run(inputs, trace=False)
    return out

